# revision 16
# baseline (speedup 1.0000x reference)
"""Multi-head attention Trainium2 kernel (8 NeuronCores, SPMD).

Problem: nn_MultiHeadAttention (B=2, S=2048, D=768, H=12, d_k=64), f32 I/O.

Sharding: 24 (batch, head) pairs -> 8 cores x 3 heads. Core c handles
batch b = c // 4 and heads [3*(c%4), 3*(c%4)+3). 4-core ReduceScatter
per batch leaves each core a distinct 512-row slice; host concatenates.

v2 structure (vs. the single-packed baseline):
- scores matmuls run 2-up in the PE array via row tiling: the
  contraction is d_k=64 (half the array), so heads (0,1) pair up on
  partition halves of the group-0 projection tiles, and head 2 pairs
  with itself across two q-blocks using an SBUF->SBUF DMA duplicate of
  its q/k rows on partitions 64-127.
- PV matmuls for heads 0,1 run 2-up via column tiling ([v0|v1] = 128
  output rows exactly); their softmax denominators come from a 4-up
  column-tiled ones-matmul round over the exp tiles (ones stationary is
  [128,1] per strip -> negligible LDWEIGHTS, one PSUM bank total).
  Head 2 keeps the 65-wide ones-column form (same slot count as
  packing would give with an odd head out).
- one ACT exp instruction per scores round covers both packed heads
  (strided out AP into the shared [128,16,1024] exp tile).
- softmax reciprocal broadcast across partitions runs on the otherwise
  idle GPSIMD engine (partition_broadcast), not a DRAM DMA bounce.
- attention is emitted q-block-major; the output projection +
  ReduceScatter of each 512-row group is emitted as soon as its
  q-block's three heads finish, hiding out-proj and collectives under
  the exp stream (ACT is the steady-state bottleneck at ~89us).
- V-proj bias add is one strided DVE op per 128-row chunk.
"""

import numpy as np
import ml_dtypes

B = 2
S = 2048
D = 768
H = 12
DK = 64
HPC = 3           # heads per core
HD = HPC * DK     # 192 head-feature columns per core
NCORES = 8
GROUP = 4         # cores per batch (reduce-scatter group)
QS = S // GROUP   # 512 output rows per core

_compiled = None


def _build(reps=1, collective=True):
    """Build the SPMD program. reps>1 emits the whole pipeline N times
    back-to-back (same inputs/outputs) - used only for timing, where
    (T_reps - T_1)/(reps-1) cancels the per-dispatch overhead.
    collective=False drops the final ReduceScatter (for TimelineSim)."""
    import concourse.mybir as mybir
    import concourse.tile as tile
    from concourse import bacc
    import concourse.bass as bass
    from concourse.bass import ts
    import contextlib

    bf16 = mybir.dt.bfloat16
    f32 = mybir.dt.float32

    nc = bacc.Bacc(num_devices=NCORES)

    qt = nc.dram_tensor("qt", [D, S], bf16, kind="ExternalInput")
    kt = nc.dram_tensor("kt", [D, S], bf16, kind="ExternalInput")
    vt = nc.dram_tensor("vt", [D, S], bf16, kind="ExternalInput")
    wq = nc.dram_tensor("wq", [D, HD], bf16, kind="ExternalInput")
    wk = nc.dram_tensor("wk", [D, HD], bf16, kind="ExternalInput")
    wv = nc.dram_tensor("wv", [D, HD], bf16, kind="ExternalInput")
    wo = nc.dram_tensor("wo", [HD + 1, D], bf16, kind="ExternalInput")
    bq = nc.dram_tensor("bq", [HD, 1], f32, kind="ExternalInput")
    bk = nc.dram_tensor("bk", [HD, 1], f32, kind="ExternalInput")
    bv = nc.dram_tensor("bv", [1, HD], f32, kind="ExternalInput")
    out_ext = nc.dram_tensor("out", [QS, D], bf16, kind="ExternalOutput")
    out_part = nc.dram_tensor("out_part", [S, D], bf16)
    out_rs = nc.dram_tensor("out_rs", [QS, D], bf16)

    RGROUPS = [list(range(g * GROUP, (g + 1) * GROUP))
               for g in range(NCORES // GROUP)]
    NC_ = D // 128      # 6 contraction chunks for the projections
    NKC = S // 128      # 16 kv chunks
    NQB = S // 512      # 4 q blocks
    SCALE = float(1.0 / np.sqrt(DK))

    with tile.TileContext(nc) as tc:
      with (tc.For_i(0, reps, 1) if reps > 1 else contextlib.nullcontext()):
       with contextlib.ExitStack() as ctx:
        consts = ctx.enter_context(tc.tile_pool(name="consts", bufs=1))
        nrm_pool = ctx.enter_context(tc.tile_pool(name="nrm", bufs=4))
        # eA tiles get their own pool allocated below the qt/kt region so
        # the first q-block's exp is not serialized behind the g1
        # projection (the big sm pool reuses qt/kt's freed space).
        sm_early = ctx.enter_context(tc.tile_pool(name="sme", bufs=1))
        fo_pool = ctx.enter_context(tc.tile_pool(name="fo", bufs=2))
        # PSUM budget (8 banks): scores pair tile [128,4,512] = 4,
        # acc (QKV proj / PV / d-round) 2x[128,512] = 2, out-proj
        # [128,768] = 2.
        sc_psum = ctx.enter_context(
            tc.tile_pool(name="scp", bufs=1, space="PSUM"))
        acc_psum = ctx.enter_context(
            tc.tile_pool(name="accp", bufs=2, space="PSUM"))
        fo_psum = ctx.enter_context(
            tc.tile_pool(name="fop", bufs=1, space="PSUM"))

        ins_ctx = contextlib.ExitStack()
        ins_pool = ins_ctx.enter_context(tc.tile_pool(name="ins", bufs=1))

        # ---- load inputs, in consumption order, alternating HWDGE queues
        dmae = [nc.sync, nc.scalar]
        ins_sb, w_sb, bias_sb = {}, {}, {}

        def load_w(name, t):
            sb = consts.tile([128, NC_, HD], bf16, tag=name)
            nc.scalar.dma_start(
                out=sb, in_=t[:, :].rearrange("(c p) n -> p c n", p=128))
            w_sb[name] = sb

        def load_bias(name, t):
            b0 = consts.tile([128, 1], f32, tag=name + "0")
            nc.sync.dma_start(out=b0, in_=t[0:128, :])
            b1 = consts.tile([HD - 128, 1], f32, tag=name + "1")
            nc.sync.dma_start(out=b1, in_=t[128:HD, :])
            bias_sb[name] = (b0, b1)

        def load_in(name, t, di=[0], pool=None):
            # vt lives in consts (needed past the qt/kt pool close; its
            # 24 KB/partition is affordable for the whole kernel)
            sb = (pool or ins_pool).tile([128, NC_, S], bf16, tag=name,
                                         name=name)
            for c in range(NC_):
                dmae[di[0] % 2].dma_start(
                    out=sb[:, c, :], in_=t[c * 128:(c + 1) * 128, :])
                di[0] += 1
            ins_sb[name] = sb

        def load_in_slabs(name, t, eng):
            # kt/qt arrive as 512-column slabs (all 6 feature chunks per
            # slab) so the per-q-block projections start as data lands;
            # kt rides the SP queue and qt the ACT queue, concurrently.
            sb = ins_pool.tile([128, NC_, S], bf16, tag=name, name=name)
            for qb in range(NQB):
                for c in range(NC_):
                    eng.dma_start(
                        out=sb[:, c, ts(qb, 512)],
                        in_=t[c * 128:(c + 1) * 128, ts(qb, 512)])
            ins_sb[name] = sb

        load_w("wk", wk)
        load_bias("bk", bk)
        load_w("wq", wq)
        load_bias("bq", bq)
        load_in_slabs("kt", kt, nc.sync)
        load_in_slabs("qt", qt, nc.scalar)
        load_w("wv", wv)
        bv_bc = consts.tile([128, HD], f32, tag="bv")
        nc.sync.dma_start(
            out=bv_bc,
            in_=bass.AP(tensor=bv[:, :].tensor, offset=bv[:, :].offset,
                        ap=[[0, 128]] + bv[:, :].ap[1:]))
        load_in("vt", vt, pool=consts)
        wo0 = consts.tile([128, D], bf16, tag="wo0")
        nc.scalar.dma_start(out=wo0, in_=wo[0:128, :])
        wo1 = consts.tile([HD + 1 - 128, D], bf16, tag="wo1")
        nc.scalar.dma_start(out=wo1, in_=wo[128:HD + 1, :])
        # Touch the exp table early so ACT's table DMA overlaps the loads.
        warm = consts.tile([1, 1], f32, tag="warm")
        nc.vector.memset(warm, 0.0)
        nc.scalar.activation(out=warm, in_=warm,
                             func=mybir.ActivationFunctionType.Exp)

        # ---- persistent activation-layout tiles ----
        # group 0: heads 0,1 on partition halves; group 1: head 2 on
        # 0-63 with a duplicate on 64-127 (for self-paired scores).
        proj = {
            (n, g): consts.tile([128, S], bf16, tag=f"{n}T{g}",
                                name=f"{n}T{g}")
            for n in ("q", "k") for g in (0, 1)
        }
        v_sb = consts.tile([128, NKC, HD + 1], bf16, tag="v")
        nc.vector.memset(v_sb[:, :, HD:HD + 1], 1.0)  # head-2 ones col
        outT0 = consts.tile([128, S], bf16, tag="outT0")
        outT1 = consts.tile([DK + 1, S], bf16, tag="outT1")
        nc.vector.memset(outT1[DK:DK + 1, :], 1.0)    # out-proj bias row
        ones4 = consts.tile([128, 4], bf16, tag="ones4")
        nc.vector.memset(ones4, 1.0)

        # ---- Q/K projections ----
        def emit_qk_proj(gi):
            if gi == 0:
                # qb-major so each projection block starts on slab arrival
                for qb in range(NQB):
                    for name, wname, bname in (("k", "wk", "bk"),
                                               ("q", "wq", "bq")):
                        x_sb = ins_sb["kt" if name == "k" else "qt"]
                        dest = proj[(name, 0)]
                        b0 = bias_sb[bname][0]
                        ps = acc_psum.tile([128, 512], f32, tag="acc")
                        for c in range(NC_):
                            nc.tensor.matmul(
                                ps, lhsT=w_sb[wname][:, c, 0:128],
                                rhs=x_sb[:, c, ts(qb, 512)],
                                start=(c == 0), stop=(c == NC_ - 1))
                        nc.vector.tensor_scalar_add(
                            out=dest[:, ts(qb, 512)], in0=ps, scalar1=b0)
            else:
                # head 2: q on PE column-groups 0-1, k on 2-3, concurrent
                for qb in range(NQB):
                    ps = acc_psum.tile([128, 512], f32, tag="acc")
                    for c in range(NC_):
                        nc.tensor.matmul(
                            ps[0:64, :], lhsT=w_sb["wq"][:, c, 128:192],
                            rhs=ins_sb["qt"][:, c, ts(qb, 512)],
                            start=(c == 0), stop=(c == NC_ - 1),
                            tile_position=(0, 0))
                        nc.tensor.matmul(
                            ps[64:128, :], lhsT=w_sb["wk"][:, c, 128:192],
                            rhs=ins_sb["kt"][:, c, ts(qb, 512)],
                            start=(c == 0), stop=(c == NC_ - 1),
                            tile_position=(0, 64))
                    nc.vector.tensor_scalar_add(
                        out=proj[("q", 1)][0:64, ts(qb, 512)],
                        in0=ps[0:64, :], scalar1=bias_sb["bq"][1])
                    nc.vector.tensor_scalar_add(
                        out=proj[("k", 1)][0:64, ts(qb, 512)],
                        in0=ps[64:128, :], scalar1=bias_sb["bk"][1])

        emit_qk_proj(0)

        # ---- attention building blocks ----
        def emit_scores_pair(lhsA, rhsA, lhsB, rhsB, pool):
            """8 rounds x 2 kv-chunks; A on PE rows 0-63, B on rows
            64-127 (row tiling, concurrent). One exp per round covers
            both. Returns exp tile [128, NKC, 1024], A in cols 0-511."""
            ep = pool.tile([128, NKC, 1024], bf16, tag="ep")
            epf = ep[:, :, :]
            for r in range(8):
                scp = sc_psum.tile([128, 4, 512], f32, tag="sc")
                for j in range(2):
                    kc = 2 * r + j
                    nc.tensor.matmul(scp[:, j, :],
                                     lhsT=lhsA[:, ts(kc, 128)], rhs=rhsA,
                                     start=True, stop=True)
                    nc.tensor.matmul(scp[:, 2 + j, :],
                                     lhsT=lhsB[:, ts(kc, 128)], rhs=rhsB,
                                     start=True, stop=True)
                # out iterates (head-half, kc, q) to match scp's slot order
                out_ap = bass.AP(
                    tensor=epf.tensor, offset=epf.offset + (2 * r) * 1024,
                    ap=[epf.ap[0], [512, 2], [1024, 2], [1, 512]])
                nc.scalar.activation(out=out_ap, in_=scp[:, :, :],
                                     func=mybir.ActivationFunctionType.Exp,
                                     scale=SCALE)
            return ep

        def emit_pv_pair(ep):
            """heads 0,1 PV 2-up via column tiling; returns psum
            [128,512]: rows 0-63 = h0 dims, 64-127 = h1 dims."""
            acc = acc_psum.tile([128, 512], f32, tag="acc")
            for kc in range(NKC):
                nc.tensor.matmul(acc[0:64, :], lhsT=v_sb[:, kc, 0:64],
                                 rhs=ep[:, kc, 0:512],
                                 start=(kc == 0), stop=(kc == NKC - 1))
                nc.tensor.matmul(acc[64:128, :], lhsT=v_sb[:, kc, 64:128],
                                 rhs=ep[:, kc, 512:1024],
                                 start=(kc == 0), stop=(kc == NKC - 1))
            return acc

        def emit_d_round(streams):
            """softmax denominators for up to 4 (exp-tile, col) streams,
            4-up column-tiled; d_j lands on partition 32j of one bank."""
            accd = acc_psum.tile([128, 512], f32, tag="acc")
            for j, (ep, co) in enumerate(streams):
                for kc in range(NKC):
                    nc.tensor.matmul(accd[32 * j:32 * j + 1, :],
                                     lhsT=ones4[:, j:j + 1],
                                     rhs=ep[:, kc, co:co + 512],
                                     start=(kc == 0), stop=(kc == NKC - 1),
                                     tile_position=(0, 32 * j))
            return accd

        def emit_norm(dst, pv_ap, d_ap):
            recip = nrm_pool.tile([1, 512], f32, tag="recip")
            nc.vector.reciprocal(recip, d_ap)
            rbc = nrm_pool.tile([64, 512], f32, tag="rbc")
            nc.gpsimd.partition_broadcast(rbc, recip[0:1, :])
            nc.vector.tensor_mul(dst, pv_ap, rbc)

        def emit_pv2(ep, co, qb):
            """head-2 PV, 65-wide with built-in denominator row."""
            acc = acc_psum.tile([128, 512], f32, tag="acc")
            for kc in range(NKC):
                nc.tensor.matmul(acc[0:DK + 1, :],
                                 lhsT=v_sb[:, kc, 2 * DK:HD + 1],
                                 rhs=ep[:, kc, co:co + 512],
                                 start=(kc == 0), stop=(kc == NKC - 1))
            emit_norm(outT1[0:DK, ts(qb, 512)], acc[0:DK, :],
                      acc[DK:DK + 1, :])

        def emit_v_proj():
            for st in range(NKC):
                ps = acc_psum.tile([128, 512], f32, tag="acc")
                for c in range(NC_):
                    nc.tensor.matmul(
                        ps[:, 0:HD], lhsT=ins_sb["vt"][:, c, ts(st, 128)],
                        rhs=w_sb["wv"][:, c, :],
                        start=(c == 0), stop=(c == NC_ - 1))
                nc.vector.tensor_add(v_sb[:, st, 0:HD], ps[:, 0:HD], bv_bc)

        def emit_outproj(og):
            ot = fo_pool.tile([128, 4, D], bf16, tag="ot")
            for sq in range(4):
                qt_ = og * 4 + sq
                ps = fo_psum.tile([128, D], f32, tag="fo")
                for noff, nsz in ((0, 512), (512, 256)):
                    nc.tensor.matmul(
                        ps[:, noff:noff + nsz],
                        lhsT=outT0[:, ts(qt_, 128)],
                        rhs=wo0[:, noff:noff + nsz],
                        start=True, stop=False)
                    nc.tensor.matmul(
                        ps[:, noff:noff + nsz],
                        lhsT=outT1[:, ts(qt_, 128)],
                        rhs=wo1[:, noff:noff + nsz],
                        start=False, stop=True)
                nc.vector.tensor_copy(out=ot[:, sq, :], in_=ps)
            nc.sync.dma_start(
                out=out_part[:, :].rearrange(
                    "(g t p) d -> g p t d", p=128, t=4)[og],
                in_=ot)
            if collective:
                nc.gpsimd.collective_compute(
                    "ReduceScatter", mybir.AluOpType.add,
                    replica_groups=RGROUPS,
                    ins=[out_part[ts(og, 512), :]],
                    outs=[out_rs[ts(og, 128), :]])
            nc.sync.dma_start(out=out_ext[ts(og, 128), :],
                              in_=(out_rs if collective
                                   else out_part)[ts(og, 128), :])

        # ---- q-block-major attention with interleaved out-proj ----
        k0, q0 = proj[("k", 0)], proj[("q", 0)]
        k1, q1 = proj[("k", 1)], proj[("q", 1)]
        # first q-block's scores go ahead of the g1 projection
        e_first = emit_scores_pair(k0[0:64, :], q0[0:64, ts(0, 512)],
                                   k0[64:128, :], q0[64:128, ts(0, 512)],
                                   pool=sm_early)
        emit_qk_proj(1)
        # duplicate head-2 q/k onto partitions 64-127 (DVE cannot cross
        # partitions; SBUF->SBUF DMA can)
        nc.sync.dma_start(out=proj[("q", 1)][64:128, :],
                          in_=proj[("q", 1)][0:64, :])
        nc.sync.dma_start(out=proj[("k", 1)][64:128, :],
                          in_=proj[("k", 1)][0:64, :])
        ins_ctx.close()   # free qt/kt SBUF before the exp pool reserves
        sm_pool = ctx.enter_context(tc.tile_pool(name="sm", bufs=2))

        first = True
        for a in (0, 2):
            b = a + 1
            eA = e_first if a == 0 else emit_scores_pair(
                k0[0:64, :], q0[0:64, ts(a, 512)],
                k0[64:128, :], q0[64:128, ts(a, 512)], pool=sm_early)
            eB = emit_scores_pair(k0[0:64, :], q0[0:64, ts(b, 512)],
                                  k0[64:128, :], q0[64:128, ts(b, 512)],
                                  pool=sm_pool)
            e2 = emit_scores_pair(k1[0:64, :], q1[0:64, ts(a, 512)],
                                  k1[64:128, :], q1[64:128, ts(b, 512)],
                                  pool=sm_pool)
            if first:
                emit_v_proj()
                first = False
            pvA = emit_pv_pair(eA)
            accd = emit_d_round([(eA, 0), (eA, 512), (eB, 0), (eB, 512)])
            pvB = emit_pv_pair(eB)
            emit_norm(outT0[0:64, ts(a, 512)], pvA[0:64, :], accd[0:1, :])
            emit_norm(outT0[64:128, ts(a, 512)], pvA[64:128, :],
                      accd[32:33, :])
            emit_norm(outT0[0:64, ts(b, 512)], pvB[0:64, :], accd[64:65, :])
            emit_norm(outT0[64:128, ts(b, 512)], pvB[64:128, :],
                      accd[96:97, :])
            emit_pv2(e2, 0, a)
            emit_pv2(e2, 512, b)
            emit_outproj(a)
            emit_outproj(b)

    nc.compile()
    return nc


def _get_compiled():
    global _compiled
    if _compiled is None:
        _compiled = _build()
    return _compiled


def make_in_maps(q, k, v, Wq, bq, Wk, bk, Wv, bv, Wo, bo):
    bf = ml_dtypes.bfloat16
    in_maps = []
    for c in range(NCORES):
        b = c // GROUP
        g = c % GROUP
        cols = slice(g * HD, (g + 1) * HD)   # head-feature columns
        wo_aug = np.empty((HD + 1, D), np.float32)
        wo_aug[:HD] = Wo.T[cols.start:cols.stop, :]
        wo_aug[HD] = bo / GROUP              # summed GROUP times by the RS
        in_maps.append({
            "qt": np.ascontiguousarray(q[b].T).astype(bf),
            "kt": np.ascontiguousarray(k[b].T).astype(bf),
            "vt": np.ascontiguousarray(v[b].T).astype(bf),
            "wq": np.ascontiguousarray(Wq.T[:, cols]).astype(bf),
            "wk": np.ascontiguousarray(Wk.T[:, cols]).astype(bf),
            "wv": np.ascontiguousarray(Wv.T[:, cols]).astype(bf),
            "wo": wo_aug.astype(bf),
            "bq": np.ascontiguousarray(bq[cols].reshape(HD, 1)).astype(np.float32),
            "bk": np.ascontiguousarray(bk[cols].reshape(HD, 1)).astype(np.float32),
            "bv": np.ascontiguousarray(bv[cols].reshape(1, HD)).astype(np.float32),
        })
    return in_maps


def kernel(q, k, v, Wq, bq, Wk, bk, Wv, bv, Wo, bo):
    from concourse.bass_utils import run_bass_kernel_spmd

    q = np.asarray(q, np.float32)
    k = np.asarray(k, np.float32)
    v = np.asarray(v, np.float32)
    nc = _get_compiled()
    in_maps = make_in_maps(q, k, v,
                           np.asarray(Wq, np.float32), np.asarray(bq, np.float32),
                           np.asarray(Wk, np.float32), np.asarray(bk, np.float32),
                           np.asarray(Wv, np.float32), np.asarray(bv, np.float32),
                           np.asarray(Wo, np.float32), np.asarray(bo, np.float32))
    res = run_bass_kernel_spmd(nc, in_maps, list(range(NCORES))).results
    out = np.empty((B, S, D), np.float32)
    for c in range(NCORES):
        b = c // GROUP
        j = c % GROUP
        # chunked reduce-scatter: chunk g of core (b, j) holds batch-b
        # rows [512*g + 128*j, 512*g + 128*j + 128)
        chunks = res[c]["out"].reshape(GROUP, 128, D)
        for g in range(GROUP):
            out[b, 512 * g + 128 * j:512 * g + 128 * j + 128, :] = chunks[g]
    return out


# revision 38
# speedup vs baseline: 1.0897x; 1.0897x over previous
"""Multi-head attention Trainium2 kernel (8 NeuronCores, SPMD).

Problem: nn_MultiHeadAttention (B=2, S=2048, D=768, H=12, d_k=64), f32 I/O.

Sharding: 24 (batch, head) pairs -> 8 cores x 3 heads. Core c handles
batch b = c // 4 and heads [3*(c%4), 3*(c%4)+3). 4-core ReduceScatter
per batch leaves each core a distinct 512-row slice; host concatenates.

v3 structure (vs. the baseline):
- the scalar engine's exp stream is the critical resource (~93us);
  everything is arranged to keep it saturated: scores PSUM is
  double-buffered (2x[128,2,512]) so exp(round r) overlaps the
  matmuls of round r+1, and exp starts ~6us in thanks to slab-wise
  q/k loads (512-column slabs -> per-q-block projections begin as
  data lands, kt on the SP DMA queue, qt on the ACT queue).
- attention is emitted q-block-major; the output projection +
  ReduceScatter of each 512-row group is emitted as soon as its
  q-block's three heads finish, hiding out-proj and collectives under
  the exp stream instead of a ~20us serial tail.
- softmax reciprocal broadcast across partitions runs on the otherwise
  idle GPSIMD engine (partition_broadcast), not a DRAM DMA bounce.
- the first q-block's exp tile lives in its own small pool placed
  below the qt/kt region, so the first exps are not serialized behind
  the group-1 projection (the main exp pool reuses qt/kt's space).
- V-proj bias add is one strided DVE op per 128-row chunk.
- (tile_position packing of the 64-contraction scores matmuls was
  tried and measured NOT concurrent on this hardware path - the HW
  time tracks the serial cost model - so attention is unpacked.)
"""

import numpy as np
import ml_dtypes

B = 2
S = 2048
D = 768
H = 12
DK = 64
HPC = 3           # heads per core
HD = HPC * DK     # 192 head-feature columns per core
NCORES = 8
GROUP = 4         # cores per batch (reduce-scatter group)
QS = S // GROUP   # 512 output rows per core

_compiled = None


def _build(reps=1, collective=True):
    """Build the SPMD program. reps>1 emits the whole pipeline N times
    back-to-back (same inputs/outputs) - used only for timing, where
    (T_reps - T_1)/(reps-1) cancels the per-dispatch overhead.
    collective=False drops the final ReduceScatter (for TimelineSim)."""
    import concourse.mybir as mybir
    import concourse.tile as tile
    from concourse import bacc
    import concourse.bass as bass
    from concourse.bass import ts
    import contextlib

    bf16 = mybir.dt.bfloat16
    f32 = mybir.dt.float32

    nc = bacc.Bacc(num_devices=NCORES)

    qt = nc.dram_tensor("qt", [D, S], bf16, kind="ExternalInput")
    kt = nc.dram_tensor("kt", [D, S], bf16, kind="ExternalInput")
    vt = nc.dram_tensor("vt", [D, S], bf16, kind="ExternalInput")
    wq = nc.dram_tensor("wq", [D, HD], bf16, kind="ExternalInput")
    wk = nc.dram_tensor("wk", [D, HD], bf16, kind="ExternalInput")
    wv = nc.dram_tensor("wv", [D, HD], bf16, kind="ExternalInput")
    wo = nc.dram_tensor("wo", [HD + 1, D], bf16, kind="ExternalInput")
    bq = nc.dram_tensor("bq", [HD, 1], f32, kind="ExternalInput")
    bk = nc.dram_tensor("bk", [HD, 1], f32, kind="ExternalInput")
    bv = nc.dram_tensor("bv", [1, HD], f32, kind="ExternalInput")
    out_ext = nc.dram_tensor("out", [QS, D], bf16, kind="ExternalOutput")
    out_part = nc.dram_tensor("out_part", [S, D], bf16)
    out_rs = nc.dram_tensor("out_rs", [QS, D], bf16)

    RGROUPS = [list(range(g * GROUP, (g + 1) * GROUP))
               for g in range(NCORES // GROUP)]
    NC_ = D // 128      # 6 contraction chunks for the projections
    NKC = S // 128      # 16 kv chunks
    NQB = S // 512      # 4 q blocks
    VW = DK + 2         # 66-wide per-head V block: 64 dims + ones col + pad
    SCALE = float(1.0 / np.sqrt(DK))

    with tile.TileContext(nc) as tc:
      with (tc.For_i(0, reps, 1) if reps > 1 else contextlib.nullcontext()):
       with contextlib.ExitStack() as ctx:
        consts = ctx.enter_context(tc.tile_pool(name="consts", bufs=1))
        nrm_pool = ctx.enter_context(tc.tile_pool(name="nrm", bufs=4))
        # q-block-0 exp tiles get their own pool allocated below the
        # qt/kt region so the first exps are not serialized behind the
        # g1 projection (the big sm pool reuses qt/kt's freed space).
        sm_early = ctx.enter_context(tc.tile_pool(name="sme", bufs=3))
        fo_pool = ctx.enter_context(tc.tile_pool(name="fo", bufs=2))
        # PSUM budget (8 banks): scores 2x[128,2,512] double-buffered = 4,
        # acc (QKV proj / PV) 2x[128,512] = 2, out-proj [128,768] = 2.
        sc_psum = ctx.enter_context(
            tc.tile_pool(name="scp", bufs=2, space="PSUM"))
        acc_psum = ctx.enter_context(
            tc.tile_pool(name="accp", bufs=2, space="PSUM"))
        fo_psum = ctx.enter_context(
            tc.tile_pool(name="fop", bufs=1, space="PSUM"))

        ins_ctx = contextlib.ExitStack()
        ins_pool = ins_ctx.enter_context(tc.tile_pool(name="ins", bufs=1))

        # ---- load inputs, in consumption order ----
        ins_sb, w_sb, bias_sb = {}, {}, {}

        def load_w(name, t):
            sb = consts.tile([128, NC_, HD], bf16, tag=name)
            nc.scalar.dma_start(
                out=sb, in_=t[:, :].rearrange("(c p) n -> p c n", p=128))
            w_sb[name] = sb

        def load_bias(name, t):
            b0 = consts.tile([128, 1], f32, tag=name + "0")
            nc.sync.dma_start(out=b0, in_=t[0:128, :])
            b1 = consts.tile([HD - 128, 1], f32, tag=name + "1")
            nc.sync.dma_start(out=b1, in_=t[128:HD, :])
            bias_sb[name] = (b0, b1)

        def load_in(name, t, pool=None):
            # vt lives in consts (needed past the qt/kt pool close; its
            # 24 KB/partition is affordable for the whole kernel)
            sb = (pool or ins_pool).tile([128, NC_, S], bf16, tag=name,
                                         name=name)
            for c in range(NC_):
                nc.sync.dma_start(
                    out=sb[:, c, :], in_=t[c * 128:(c + 1) * 128, :])
            ins_sb[name] = sb

        def load_in_slabs(name, t, engs):
            # kt/qt arrive as 512-column slabs (all 6 feature chunks per
            # slab) so the per-q-block projections start as data lands.
            # engs: one DMA-issuing engine per slab - DMA issues occupy
            # the issuing engine's queue, so late-needed slabs must not
            # sit ahead of exp work in the ACT queue.
            sb = ins_pool.tile([128, NC_, S], bf16, tag=name, name=name)
            for qb in range(NQB):
                for c in range(NC_):
                    engs[qb].dma_start(
                        out=sb[:, c, ts(qb, 512)],
                        in_=t[c * 128:(c + 1) * 128, ts(qb, 512)])
            ins_sb[name] = sb

        # sync queue: biases, kt slabs, bv, vt-half; scalar queue: wk/wq,
        # qt slabs, wv, vt-half, wo. Queues drain concurrently; each DMA
        # costs ~0.6us of queue-issue time, so order = arrival time.
        load_bias("bk", bk)
        load_bias("bq", bq)
        load_w("wk", wk)
        load_w("wq", wq)
        load_in_slabs("kt", kt, [nc.sync] * 4)
        load_in_slabs("qt", qt, [nc.scalar] + [nc.gpsimd] * 3)
        # everything below rides SP: the ACT queue must be free for exp
        # after the first qt slab (DMA issues occupy the issuing engine)
        bv_bc = consts.tile([128, HD], f32, tag="bv")
        nc.sync.dma_start(
            out=bv_bc,
            in_=bass.AP(tensor=bv[:, :].tensor, offset=bv[:, :].offset,
                        ap=[[0, 128]] + bv[:, :].ap[1:]))
        sbv = consts.tile([128, NC_, HD], bf16, tag="wv", name="wv")
        nc.sync.dma_start(
            out=sbv, in_=wv[:, :].rearrange("(c p) n -> p c n", p=128))
        w_sb["wv"] = sbv
        load_in("vt", vt, pool=consts)
        wo0 = consts.tile([128, D], bf16, tag="wo0")
        nc.sync.dma_start(out=wo0, in_=wo[0:128, :])
        wo1 = consts.tile([HD + 1 - 128, D], bf16, tag="wo1")
        nc.sync.dma_start(out=wo1, in_=wo[128:HD + 1, :])
        # Touch the exp table early so ACT's table DMA overlaps the loads.
        warm = consts.tile([1, 1], f32, tag="warm")
        nc.vector.memset(warm, 0.0)
        nc.scalar.activation(out=warm, in_=warm,
                             func=mybir.ActivationFunctionType.Exp)

        # ---- persistent activation-layout tiles ----
        # group 0: heads 0,1 on partition halves; group 1: head 2 on
        # 0-63 with a duplicate on 64-127 (for self-paired scores).
        proj = {
            (n, g): consts.tile([128 if g == 0 else 64, S], bf16,
                                tag=f"{n}T{g}", name=f"{n}T{g}")
            for n in ("q", "k") for g in (0, 1)
        }
        v_sb = consts.tile([128, NKC, HPC * VW], bf16, tag="v")
        for h in range(HPC):
            nc.vector.memset(v_sb[:, :, VW * h + DK:VW * h + DK + 1], 1.0)
        outT0 = consts.tile([128, S], bf16, tag="outT0")
        outT1 = consts.tile([DK + 1, S], bf16, tag="outT1")
        nc.vector.memset(outT1[DK:DK + 1, :], 1.0)    # out-proj bias row

        # ---- Q/K projections ----
        def g0_block(name, qb):
            wname, bname = ("wk", "bk") if name == "k" else ("wq", "bq")
            x_sb = ins_sb["kt" if name == "k" else "qt"]
            ps = acc_psum.tile([128, 512], f32, tag="acc")
            for c in range(NC_):
                nc.tensor.matmul(
                    ps, lhsT=w_sb[wname][:, c, 0:128],
                    rhs=x_sb[:, c, ts(qb, 512)],
                    start=(c == 0), stop=(c == NC_ - 1))
            nc.vector.tensor_scalar_add(
                out=proj[(name, 0)][:, ts(qb, 512)], in0=ps,
                scalar1=bias_sb[bname][0])

        def emit_g1_proj():
            # head 2: q on PE column-groups 0-1, k on 2-3, concurrent
            for qb in range(NQB):
                ps = acc_psum.tile([128, 512], f32, tag="acc")
                for c in range(NC_):
                    nc.tensor.matmul(
                        ps[0:64, :], lhsT=w_sb["wq"][:, c, 128:192],
                        rhs=ins_sb["qt"][:, c, ts(qb, 512)],
                        start=(c == 0), stop=(c == NC_ - 1),
                        tile_position=(0, 0))
                    nc.tensor.matmul(
                        ps[64:128, :], lhsT=w_sb["wk"][:, c, 128:192],
                        rhs=ins_sb["kt"][:, c, ts(qb, 512)],
                        start=(c == 0), stop=(c == NC_ - 1),
                        tile_position=(0, 64))
                nc.vector.tensor_scalar_add(
                    out=proj[("q", 1)][0:64, ts(qb, 512)],
                    in0=ps[0:64, :], scalar1=bias_sb["bq"][1])
                nc.vector.tensor_scalar_add(
                    out=proj[("k", 1)][0:64, ts(qb, 512)],
                    in0=ps[64:128, :], scalar1=bias_sb["bk"][1])

        # ---- attention building blocks ----
        def scores_rounds(ep, kth, qth, rr):
            """rounds of 2 kv-chunks; scores PSUM double-buffered so
            exp(r) overlaps the matmuls of round r+1."""
            for r in rr:
                scp = sc_psum.tile([128, 2, 512], f32, tag="sc")
                for j in range(2):
                    nc.tensor.matmul(scp[:, j, :],
                                     lhsT=kth[:, ts(2 * r + j, 128)],
                                     rhs=qth, start=True, stop=True)
                nc.scalar.activation(out=ep[:, 2 * r:2 * r + 2, :],
                                     in_=scp[:, :, :],
                                     func=mybir.ActivationFunctionType.Exp,
                                     scale=SCALE)

        def emit_scores(h, qb, pool):
            ep = pool.tile([128, NKC, 512], bf16, tag="ep", name="ep")
            scores_rounds(ep, *head_slices(h, qb), range(8))
            return ep

        def emit_norm(dst, pv_ap, d_ap):
            recip = nrm_pool.tile([1, 512], f32, tag="recip")
            nc.vector.reciprocal(recip, d_ap)
            rbc = nrm_pool.tile([64, 512], f32, tag="rbc")
            nc.gpsimd.partition_broadcast(rbc, recip[0:1, :])
            nc.vector.tensor_mul(dst, pv_ap, rbc)

        def emit_pv(ep, h, qb):
            """65-wide PV (64 dims + ones column -> denominator row)."""
            acc = acc_psum.tile([128, 512], f32, tag="acc")
            for kc in range(NKC):
                nc.tensor.matmul(acc[0:DK + 1, :],
                                 lhsT=v_sb[:, kc, VW * h:VW * h + DK + 1],
                                 rhs=ep[:, kc, :],
                                 start=(kc == 0), stop=(kc == NKC - 1))
            dst = (outT0[ts(h, DK), ts(qb, 512)] if h < 2
                   else outT1[0:DK, ts(qb, 512)])
            emit_norm(dst, acc[0:DK, :], acc[DK:DK + 1, :])

        def emit_v_proj():
            # natural-layout V in 66-stride head blocks (dims + ones col)
            vf = v_sb[:, 0, :]
            for st in range(NKC):
                ps = acc_psum.tile([128, 512], f32, tag="acc")
                for c in range(NC_):
                    nc.tensor.matmul(
                        ps[:, 0:HD], lhsT=ins_sb["vt"][:, c, ts(st, 128)],
                        rhs=w_sb["wv"][:, c, :],
                        start=(c == 0), stop=(c == NC_ - 1))
                # one strided add: [3 heads x 64 dims] at stride VW
                out_ap = bass.AP(tensor=vf.tensor,
                                 offset=vf.offset + st * (HPC * VW),
                                 ap=[vf.ap[0], [VW, HPC], [1, DK]])
                nc.vector.tensor_add(out_ap, ps[:, 0:HD], bv_bc)

        def emit_outproj(og):
            ot = fo_pool.tile([128, 4, D], bf16, tag="ot")
            for sq in range(4):
                qt_ = og * 4 + sq
                ps = fo_psum.tile([128, D], f32, tag="fo")
                for noff, nsz in ((0, 512), (512, 256)):
                    nc.tensor.matmul(
                        ps[:, noff:noff + nsz],
                        lhsT=outT0[:, ts(qt_, 128)],
                        rhs=wo0[:, noff:noff + nsz],
                        start=True, stop=False)
                    nc.tensor.matmul(
                        ps[:, noff:noff + nsz],
                        lhsT=outT1[:, ts(qt_, 128)],
                        rhs=wo1[:, noff:noff + nsz],
                        start=False, stop=True)
                # nc.any: mid-kernel the scheduler routes these to DVE
                # (ACT has exp queued); the tail block's land on the
                # by-then-idle ACT engine
                nc.any.tensor_copy(out=ot[:, sq, :], in_=ps)
            nc.sync.dma_start(
                out=out_part[:, :].rearrange(
                    "(g t p) d -> g p t d", p=128, t=4)[og],
                in_=ot)
            if collective:
                nc.gpsimd.collective_compute(
                    "ReduceScatter", mybir.AluOpType.add,
                    replica_groups=RGROUPS,
                    ins=[out_part[ts(og, 512), :]],
                    outs=[out_rs[ts(og, 128), :]])
            nc.sync.dma_start(out=out_ext[ts(og, 128), :],
                              in_=(out_rs if collective
                                   else out_part)[ts(og, 128), :])

        def head_slices(h, qb):
            if h < 2:
                return (proj[("k", 0)][ts(h, 64), :],
                        proj[("q", 0)][ts(h, 64), ts(qb, 512)])
            return (proj[("k", 1)][0:64, :],
                    proj[("q", 1)][0:64, ts(qb, 512)])

        # ---- startup: K-proj blocks interleaved with the first head's
        # scores rounds, so exp starts as soon as the first kt slabs land
        g0_block("k", 0)
        g0_block("q", 0)
        e_first = sm_early.tile([128, NKC, 512], bf16, tag="ep",
                                name="e_first")
        kth0, qth0 = head_slices(0, 0)
        scores_rounds(e_first, kth0, qth0, range(2))
        for i in (1, 2, 3):
            g0_block("k", i)
            scores_rounds(e_first, kth0, qth0, range(2 * i, 2 * i + 2))
        e_h1 = emit_scores(1, 0, sm_early)   # needs only g0 qb0 - early
        g0_block("q", 1)
        g0_block("q", 2)
        g0_block("q", 3)
        emit_g1_proj()
        ins_ctx.close()   # free qt/kt SBUF before the exp pool reserves
        sm_pool = ctx.enter_context(tc.tile_pool(name="sm", bufs=3))

        # ---- q-block-major attention, emission order = engine order:
        # qb0+qb1 scores precede the vt-gated V-projection (ACT stays
        # fed while vt streams in); out-projs sit where their waits are
        # already satisfied so they never stall the PE queue.
        eps = {0: [e_first, e_h1, emit_scores(2, 0, sm_early)],
               1: [emit_scores(h, 1, sm_pool) for h in range(HPC)]}
        emit_v_proj()
        for h in range(HPC):
            emit_pv(eps[0][h], h, 0)
        for h in range(HPC):
            emit_pv(eps[1][h], h, 1)
        eps[2] = [emit_scores(h, 2, sm_pool) for h in range(HPC)]
        emit_outproj(0)
        emit_outproj(1)
        for h in range(HPC):
            emit_pv(eps[2][h], h, 2)
        eps[3] = [emit_scores(h, 3, sm_pool) for h in range(HPC)]
        emit_outproj(2)
        for h in range(HPC):
            emit_pv(eps[3][h], h, 3)
        emit_outproj(3)

    nc.compile()
    return nc


def _get_compiled():
    global _compiled
    if _compiled is None:
        _compiled = _build()
    return _compiled


def make_in_maps(q, k, v, Wq, bq, Wk, bk, Wv, bv, Wo, bo):
    bf = ml_dtypes.bfloat16
    in_maps = []
    for c in range(NCORES):
        b = c // GROUP
        g = c % GROUP
        cols = slice(g * HD, (g + 1) * HD)   # head-feature columns
        wo_aug = np.empty((HD + 1, D), np.float32)
        wo_aug[:HD] = Wo.T[cols.start:cols.stop, :]
        wo_aug[HD] = bo / GROUP              # summed GROUP times by the RS
        in_maps.append({
            "qt": np.ascontiguousarray(q[b].T).astype(bf),
            "kt": np.ascontiguousarray(k[b].T).astype(bf),
            "vt": np.ascontiguousarray(v[b].T).astype(bf),
            "wq": np.ascontiguousarray(Wq.T[:, cols]).astype(bf),
            "wk": np.ascontiguousarray(Wk.T[:, cols]).astype(bf),
            "wv": np.ascontiguousarray(Wv.T[:, cols]).astype(bf),
            "wo": wo_aug.astype(bf),
            "bq": np.ascontiguousarray(bq[cols].reshape(HD, 1)).astype(np.float32),
            "bk": np.ascontiguousarray(bk[cols].reshape(HD, 1)).astype(np.float32),
            "bv": np.ascontiguousarray(bv[cols].reshape(1, HD)).astype(np.float32),
        })
    return in_maps


def kernel(q, k, v, Wq, bq, Wk, bk, Wv, bv, Wo, bo):
    from concourse.bass_utils import run_bass_kernel_spmd

    q = np.asarray(q, np.float32)
    k = np.asarray(k, np.float32)
    v = np.asarray(v, np.float32)
    nc = _get_compiled()
    in_maps = make_in_maps(q, k, v,
                           np.asarray(Wq, np.float32), np.asarray(bq, np.float32),
                           np.asarray(Wk, np.float32), np.asarray(bk, np.float32),
                           np.asarray(Wv, np.float32), np.asarray(bv, np.float32),
                           np.asarray(Wo, np.float32), np.asarray(bo, np.float32))
    res = run_bass_kernel_spmd(nc, in_maps, list(range(NCORES))).results
    out = np.empty((B, S, D), np.float32)
    for c in range(NCORES):
        b = c // GROUP
        j = c % GROUP
        # chunked reduce-scatter: chunk g of core (b, j) holds batch-b
        # rows [512*g + 128*j, 512*g + 128*j + 128)
        chunks = res[c]["out"].reshape(GROUP, 128, D)
        for g in range(GROUP):
            out[b, 512 * g + 128 * j:512 * g + 128 * j + 128, :] = chunks[g]
    return out
